# revision 3
# baseline (speedup 1.0000x reference)
"""Trainium2 Bass kernel: 16-head MHA (S=4096, D=1024) sharded 2 heads/core over 8 cores.

Per-core plan (heads h0=2c, h1=2c+1):
  - inputs: QT/KT/VT [D, S] bf16 (host-transposed, shared by all cores) + per-core
    weight slices pre-packed to SBUF layouts.
  - phase 1: projections.  qT/kT computed as [dk, S] on partitions 0:64 (per head),
    scaled by 1/sqrt(dk) and biased.  v computed directly in [t, j] layout ([128, 128]
    per t-block, j = 2 heads x 64) with a ones column appended per head for the
    softmax denominator.  v bias is exactly equivalent to a constant output shift
    (softmax rows sum to 1), so it is applied on the host.
  - phase 2 (fused per s-block): scoresT tiles [t=128, s<=512] = kT-block.T @ qT-block
    in PSUM; exp on ScalarE (no max subtraction needed: scores ~ N(0,1), |s| < 10);
    P tiles bf16 in SBUF; oT[65, 512] += [v|1].T @ P accumulated over t; row 64 is the
    softmax denominator l.  r = 1/l via fast-reciprocal DVE op; broadcast r across 64
    partitions with a K=1 fp32 matmul; xT = oT * r (bf16).  Output projection
    out[s,m] = sum_h xT_h.T @ WoT_h accumulated K=64 twice, copied to SBUF, DMA'd out.
  - host: sum the 8 partial outputs + bias terms.
"""

import sys

for _p in ("/opt/trn_rl_repo",):
    if _p not in sys.path:
        sys.path.insert(0, _p)

import numpy as np
import ml_dtypes

import concourse.bass as bass  # noqa: F401
import concourse.mybir as mybir
import concourse.tile as tile
from concourse import bacc
from concourse.bass_utils import run_bass_kernel_spmd

P = 128
S = 4096
D = 1024
H = 16
DK = 64
HL = 2            # heads per core
NC = 8            # cores
SB = 512          # s-block width
NSB = S // SB     # 8
TB = 128          # t-block (scores partition dim)
NTB = S // TB     # 32
DO = D // P       # 8 d-chunks
# t-block group sizes per (head, s-block); alternate 2/4 so the two PSUM score
# slots (2-bank and 4-bank) ping-pong and ScalarE exp never waits on TensorE.
GROUPS = [2, 4, 2, 4, 2, 4, 2, 4, 2, 4, 2]
assert sum(GROUPS) == NTB

F32 = mybir.dt.float32
BF16 = mybir.dt.bfloat16
BF16_NP = ml_dtypes.bfloat16


def build_nc(reps: int = 1):
    """Build the per-core Bass kernel; `reps` unrolled repetitions (for timing)."""
    from contextlib import ExitStack

    nc = bacc.Bacc("TRN2", target_bir_lowering=False, debug=False, num_devices=NC)
    qt = nc.dram_tensor("qt", [D, S], BF16, kind="ExternalInput").ap()
    kt = nc.dram_tensor("kt", [D, S], BF16, kind="ExternalInput").ap()
    vt = nc.dram_tensor("vt", [D, S], BF16, kind="ExternalInput").ap()
    wqk = nc.dram_tensor("wqk", [P, DO, 2, HL, DK], BF16, kind="ExternalInput").ap()
    wv = nc.dram_tensor("wv", [P, DO, P], BF16, kind="ExternalInput").ap()
    wot = nc.dram_tensor("wot", [DK, HL, D], BF16, kind="ExternalInput").ap()
    bqk = nc.dram_tensor("bqk", [DK, 4], F32, kind="ExternalInput").ap()
    out = nc.dram_tensor("out", [S, D], F32, kind="ExternalOutput").ap()

    qt_r = qt.rearrange("(o p) s -> p o s", p=P)
    kt_r = kt.rearrange("(o p) s -> p o s", p=P)
    vt_r = vt.rearrange("(o p) s -> p o s", p=P)
    out_r = out.rearrange("(so p) m -> p so m", p=P)  # [128, 32, 1024]

    with tile.TileContext(nc) as tc, ExitStack() as ctx:
        const = ctx.enter_context(tc.tile_pool(name="const", bufs=1))
        pers = ctx.enter_context(tc.tile_pool(name="pers", bufs=1))
        pin = ctx.enter_context(tc.tile_pool(name="pin", bufs=2))
        pP = ctx.enter_context(tc.tile_pool(name="pP", bufs=3))
        pout = ctx.enter_context(tc.tile_pool(name="pout", bufs=3))
        prr = ctx.enter_context(tc.tile_pool(name="prr", bufs=2))
        # PSUM: sA 4 banks + sB 2 banks + oT 1 bank + pv 1 bank = 8 banks
        psA = ctx.enter_context(tc.tile_pool(name="psA", bufs=1, space="PSUM"))
        psB = ctx.enter_context(tc.tile_pool(name="psB", bufs=1, space="PSUM"))
        psO = ctx.enter_context(tc.tile_pool(name="psO", bufs=1, space="PSUM"))
        psV = ctx.enter_context(tc.tile_pool(name="psV", bufs=1, space="PSUM"))

        wqk_sb = const.tile([P, DO, 2, HL, DK], BF16)
        nc.sync.dma_start(wqk_sb[:], wqk)
        wv_sb = const.tile([P, DO, P], BF16)
        nc.sync.dma_start(wv_sb[:], wv)
        wot_sb = const.tile([DK, HL, D], BF16)
        nc.sync.dma_start(wot_sb[:], wot)
        bqk_sb = const.tile([DK, 4], F32)
        nc.sync.dma_start(bqk_sb[:], bqk)
        ones_sb = const.tile([1, DK], F32)
        nc.vector.memset(ones_sb[:], 1.0)

        for _rep in range(reps):
            qT = pers.tile([DK, HL, S], BF16, tag="qT", name="qT")
            kT = pers.tile([DK, HL, S], BF16, tag="kT", name="kT")
            vx0 = pers.tile([P, NTB, DK + 1], BF16, tag="vx0", name="vx0")
            vx1 = pers.tile([P, NTB, DK + 1], BF16, tag="vx1", name="vx1")
            xT = pers.tile([DK, HL, S], BF16, tag="xT", name="xT")
            nc.vector.memset(vx0[:, :, DK], 1.0)
            nc.vector.memset(vx1[:, :, DK], 1.0)

            # ---------------- phase 1: projections ----------------
            for sb in range(NSB):
                s0 = sb * SB
                qs = pin.tile([P, DO, SB], BF16, tag="qs", name="qs")
                nc.sync.dma_start(qs[:], qt_r[:, :, s0 : s0 + SB])
                ks = pin.tile([P, DO, SB], BF16, tag="ks", name="ks")
                nc.sync.dma_start(ks[:], kt_r[:, :, s0 : s0 + SB])
                vs = pin.tile([P, DO, SB], BF16, tag="vs", name="vs")
                nc.sync.dma_start(vs[:], vt_r[:, :, s0 : s0 + SB])

                # q/k projections: psum slices [64, 512] all on partitions 0:64
                pq = psA.tile([P, 4 * SB], F32, tag="sA", name="pq")
                pk = psB.tile([P, 2 * SB], F32, tag="sB", name="pk")
                for h in range(HL):
                    pqh = pq[0:DK, h * SB : (h + 1) * SB]
                    pkh = pk[0:DK, h * SB : (h + 1) * SB]
                    for o in range(DO):
                        nc.tensor.matmul(
                            pqh, wqk_sb[:, o, 0, h, :], qs[:, o],
                            start=(o == 0), stop=(o == DO - 1),
                        )
                    for o in range(DO):
                        nc.tensor.matmul(
                            pkh, wqk_sb[:, o, 1, h, :], ks[:, o],
                            start=(o == 0), stop=(o == DO - 1),
                        )
                    # (q + bq) * 1/sqrt(dk);  (k + bk)
                    nc.vector.tensor_scalar(
                        qT[:, h, s0 : s0 + SB], pqh,
                        bqk_sb[:, h : h + 1], 0.125,
                        mybir.AluOpType.add, mybir.AluOpType.mult,
                    )
                    nc.vector.tensor_scalar(
                        kT[:, h, s0 : s0 + SB], pkh,
                        bqk_sb[:, 2 + h : 3 + h], None,
                        mybir.AluOpType.add,
                    )

                # v directly in [t, j] layout, j = h0 dk | h1 dk
                for tb in range(SB // TB):
                    pv = psV.tile([P, SB], F32, tag="pv", name="pvv")
                    pvv = pv[:, 0:P]
                    for o in range(DO):
                        nc.tensor.matmul(
                            pvv, vs[:, o, tb * TB : (tb + 1) * TB], wv_sb[:, o],
                            start=(o == 0), stop=(o == DO - 1),
                        )
                    tbg = sb * (SB // TB) + tb
                    nc.vector.tensor_copy(vx0[:, tbg, 0:DK], pvv[:, 0:DK])
                    nc.vector.tensor_copy(vx1[:, tbg, 0:DK], pvv[:, DK:P])

            # -------- phase 2: attention + output projection, fused per s-block -----
            for sb in range(NSB):
                s0 = sb * SB
                for h in range(HL):
                    vx = vx0 if h == 0 else vx1
                    oT = psO.tile([DK + 1, SB], F32, tag="oT", name="oT")
                    tb0 = 0
                    for g in GROUPS:
                        pool, width = (psB, 2 * SB) if g == 2 else (psA, 4 * SB)
                        sc = pool.tile([P, width], F32, tag=("sB" if g == 2 else "sA"), name="sc")
                        for i in range(g):
                            tb = tb0 + i
                            nc.tensor.matmul(
                                sc[:, i * SB : (i + 1) * SB],
                                kT[:, h, tb * TB : (tb + 1) * TB],
                                qT[:, h, s0 : s0 + SB],
                                start=True, stop=True,
                            )
                        pt = pP.tile([P, 4 * SB], BF16, tag="P", name="pt")
                        nc.scalar.activation(
                            pt[:, : g * SB], sc[:, : g * SB],
                            mybir.ActivationFunctionType.Exp,
                        )
                        for i in range(g):
                            tb = tb0 + i
                            nc.tensor.matmul(
                                oT, vx[:, tb, :], pt[:, i * SB : (i + 1) * SB],
                                start=(tb == 0), stop=(tb == NTB - 1),
                            )
                        tb0 += g

                    # r = 1/l ; broadcast over 64 partitions ; xT = oT * r
                    r_t = prr.tile([1, SB], F32, tag="r", name="r_t")
                    r_s = prr.tile([1, SB], F32, tag="rs", name="r_s")
                    l_t = prr.tile([1, SB], F32, tag="lt", name="l_t")
                    # custom-DVE reciprocal mis-reads PSUM operands: stage l in SBUF
                    nc.vector.tensor_copy(l_t[:], oT[DK : DK + 1, :])
                    nc.vector.reciprocal_approx_accurate(r_t[:], l_t[:], r_s[:])
                    bc = psV.tile([DK, SB], F32, tag="pv", name="bc")
                    nc.tensor.matmul(bc, ones_sb[:], r_t[:], start=True, stop=True)
                    # DVE has a single PSUM read port: stage the broadcast in SBUF
                    bc_sb = prr.tile([DK, SB], F32, tag="bcs", name="bc_sb")
                    nc.vector.tensor_copy(bc_sb[:], bc)
                    nc.vector.tensor_tensor(
                        xT[:, h, s0 : s0 + SB], oT[0:DK, :], bc_sb[:], mybir.AluOpType.mult
                    )

                # output projection for this s-block
                for si in range(SB // P):
                    so = sb * (SB // P) + si
                    for mb in range(2):
                        m0 = mb * SB
                        op = psV.tile([P, SB], F32, tag="pv", name="op")
                        for h in range(HL):
                            nc.tensor.matmul(
                                op, xT[:, h, so * P : (so + 1) * P],
                                wot_sb[:, h, m0 : m0 + SB],
                                start=(h == 0), stop=(h == HL - 1),
                            )
                        ob = pout.tile([P, SB], F32, tag="ob", name="ob")
                        nc.vector.tensor_copy(ob[:], op)
                        nc.sync.dma_start(out_r[:, so, m0 : m0 + SB], ob[:])

    nc.finalize()
    return nc


def _pack_core_inputs(c, QT, KT, VT, Wq, bq, Wk, bk, Wv, Wo):
    """Per-core input dict (core c owns heads 2c, 2c+1)."""
    h0 = HL * c
    wq = Wq[h0 : h0 + HL].reshape(HL, DO, P, DK).transpose(2, 1, 0, 3)  # [p,o,h,dk]
    wk = Wk[h0 : h0 + HL].reshape(HL, DO, P, DK).transpose(2, 1, 0, 3)
    wqk = np.stack([wq, wk], axis=2).astype(BF16_NP)  # [p, o, qk, h, dk]
    wv = (
        Wv[h0 : h0 + HL].reshape(HL, DO, P, DK).transpose(2, 1, 0, 3).reshape(P, DO, P)
    ).astype(BF16_NP)
    wot = (
        Wo[:, h0 * DK : (h0 + HL) * DK].reshape(D, HL, DK).transpose(2, 1, 0)
    ).astype(BF16_NP)  # [dk, h, m]
    bqk = np.stack(
        [bq[h0], bq[h0 + 1], bk[h0], bk[h0 + 1]], axis=1
    ).astype(np.float32)  # [64, 4]
    return {
        "qt": QT, "kt": KT, "vt": VT,
        "wqk": np.ascontiguousarray(wqk),
        "wv": np.ascontiguousarray(wv),
        "wot": np.ascontiguousarray(wot),
        "bqk": np.ascontiguousarray(bqk),
    }


def make_in_maps(Q, K, V, Wq, bq, Wk, bk, Wv, bv, Wo, bo):
    QT = np.ascontiguousarray(Q.T).astype(BF16_NP)
    KT = np.ascontiguousarray(K.T).astype(BF16_NP)
    VT = np.ascontiguousarray(V.T).astype(BF16_NP)
    return [
        _pack_core_inputs(c, QT, KT, VT, Wq, bq, Wk, bk, Wv, Wo) for c in range(NC)
    ]


def host_combine(partials, Wq, bv, Wo, bo):
    total = np.zeros((S, D), np.float32)
    for p in partials:
        total += p
    # v-bias passes through softmax exactly as +bv on the concat features
    total += bv.reshape(-1).astype(np.float32) @ Wo.T.astype(np.float32) + bo
    return total


_NC_CACHE = {}


def _get_nc(reps=1):
    if reps not in _NC_CACHE:
        _NC_CACHE[reps] = build_nc(reps)
    return _NC_CACHE[reps]


def kernel(Q, K, V, Wq, bq, Wk, bk, Wv, bv, Wo, bo):
    args = [np.asarray(x) for x in (Q, K, V, Wq, bq, Wk, bk, Wv, bv, Wo, bo)]
    Q, K, V, Wq, bq, Wk, bk, Wv, bv, Wo, bo = args
    nc = _get_nc()
    in_maps = make_in_maps(Q, K, V, Wq, bq, Wk, bk, Wv, bv, Wo, bo)
    res = run_bass_kernel_spmd(nc, in_maps, core_ids=list(range(NC)))
    partials = [res.results[c]["out"] for c in range(NC)]
    return host_combine(partials, Wq, bv, Wo, bo)


# revision 6
# speedup vs baseline: 21.7101x; 21.7101x over previous
"""Trainium2 Bass kernel: 16-head MHA (S=4096, D=1024) sharded 2 heads/core over 8 cores.

Per-core plan (heads h0=2c, h1=2c+1):
  - inputs: QT/KT/VT [D, S] bf16 (host-transposed, shared by all cores) + per-core
    weight slices pre-packed to SBUF layouts.
  - phase 1: projections.  qT/kT computed as [dk, S] on partitions 0:64 (per head),
    scaled by 1/sqrt(dk) and biased.  v computed directly in [t, j] layout ([128, 128]
    per t-block, j = 2 heads x 64) with a ones column appended per head for the
    softmax denominator.  v bias is exactly equivalent to a constant output shift
    (softmax rows sum to 1), so it is applied on the host.
  - phase 2 (fused per s-block): scoresT tiles [t=128, s<=512] = kT-block.T @ qT-block
    in PSUM; exp on ScalarE (no max subtraction needed: scores ~ N(0,1), |s| < 10);
    P tiles bf16 in SBUF; oT[65, 512] += [v|1].T @ P accumulated over t; row 64 is the
    softmax denominator l.  r = 1/l via fast-reciprocal DVE op; broadcast r across 64
    partitions with a K=1 fp32 matmul; xT = oT * r (bf16).  Output projection
    out[s,m] = sum_h xT_h.T @ WoT_h accumulated K=64 twice, copied to SBUF, DMA'd out.
  - host: sum the 8 partial outputs + bias terms.
"""

import sys

for _p in ("/opt/trn_rl_repo",):
    if _p not in sys.path:
        sys.path.insert(0, _p)

import numpy as np
import ml_dtypes

import concourse.bass as bass  # noqa: F401
import concourse.mybir as mybir
import concourse.tile as tile
from concourse import bacc
from concourse.bass_utils import run_bass_kernel_spmd

P = 128
S = 4096
D = 1024
H = 16
DK = 64
HL = 2            # heads per core
NC = 8            # cores
SB = 512          # s-block width
NSB = S // SB     # 8
TB = 128          # t-block (scores partition dim)
NTB = S // TB     # 32
DO = D // P       # 8 d-chunks
# t-block group sizes per (head, s-block); alternate 2/4 so the two PSUM score
# slots (2-bank and 4-bank) ping-pong and ScalarE exp never waits on TensorE.
GROUPS = [2, 4, 2, 4, 2, 4, 2, 4, 2, 4, 2]
assert sum(GROUPS) == NTB

F32 = mybir.dt.float32
BF16 = mybir.dt.bfloat16
BF16_NP = ml_dtypes.bfloat16


def build_nc(reps: int = 1, loop_n: int = 0):
    """Build the per-core Bass kernel.

    `reps`: unrolled repetitions of the body.  `loop_n` > 0 instead wraps the
    body in a hardware For loop with that many iterations (timing builds).
    """
    from contextlib import ExitStack

    nc = bacc.Bacc("TRN2", target_bir_lowering=False, debug=False, num_devices=NC)
    qt = nc.dram_tensor("qt", [D, S], BF16, kind="ExternalInput").ap()
    kt = nc.dram_tensor("kt", [D, S], BF16, kind="ExternalInput").ap()
    vt = nc.dram_tensor("vt", [D, S], BF16, kind="ExternalInput").ap()
    wqk = nc.dram_tensor("wqk", [P, DO, 2, HL, DK], BF16, kind="ExternalInput").ap()
    wv = nc.dram_tensor("wv", [P, DO, P], BF16, kind="ExternalInput").ap()
    wot = nc.dram_tensor("wot", [DK, HL, D], BF16, kind="ExternalInput").ap()
    bqk = nc.dram_tensor("bqk", [DK, 4], F32, kind="ExternalInput").ap()
    out = nc.dram_tensor("out", [S, D], F32, kind="ExternalOutput").ap()

    qt_r = qt.rearrange("(o p) s -> p o s", p=P)
    kt_r = kt.rearrange("(o p) s -> p o s", p=P)
    vt_r = vt.rearrange("(o p) s -> p o s", p=P)
    out_r = out.rearrange("(so p) m -> p so m", p=P)  # [128, 32, 1024]

    with tile.TileContext(nc) as tc, ExitStack() as ctx:
        const = ctx.enter_context(tc.tile_pool(name="const", bufs=1))
        pers = ctx.enter_context(tc.tile_pool(name="pers", bufs=1))
        pin = ctx.enter_context(tc.tile_pool(name="pin", bufs=2))
        pP = ctx.enter_context(tc.tile_pool(name="pP", bufs=3))
        pout = ctx.enter_context(tc.tile_pool(name="pout", bufs=3))
        prr = ctx.enter_context(tc.tile_pool(name="prr", bufs=2))
        # PSUM: sA 4 banks + sB 2 banks + oT 1 bank + pv 1 bank = 8 banks
        psA = ctx.enter_context(tc.tile_pool(name="psA", bufs=1, space="PSUM"))
        psB = ctx.enter_context(tc.tile_pool(name="psB", bufs=1, space="PSUM"))
        psO = ctx.enter_context(tc.tile_pool(name="psO", bufs=1, space="PSUM"))
        psV = ctx.enter_context(tc.tile_pool(name="psV", bufs=1, space="PSUM"))

        wqk_sb = const.tile([P, DO, 2, HL, DK], BF16)
        nc.sync.dma_start(wqk_sb[:], wqk)
        wv_sb = const.tile([P, DO, P], BF16)
        nc.sync.dma_start(wv_sb[:], wv)
        wot_sb = const.tile([DK, HL, D], BF16)
        nc.sync.dma_start(wot_sb[:], wot)
        bqk_sb = const.tile([DK, 4], F32)
        nc.sync.dma_start(bqk_sb[:], bqk)
        ones_sb = const.tile([1, DK], F32)
        nc.vector.memset(ones_sb[:], 1.0)

        def body():
            qT = pers.tile([DK, HL, S], BF16, tag="qT", name="qT")
            kT = pers.tile([DK, HL, S], BF16, tag="kT", name="kT")
            vx0 = pers.tile([P, NTB, DK + 1], BF16, tag="vx0", name="vx0")
            vx1 = pers.tile([P, NTB, DK + 1], BF16, tag="vx1", name="vx1")
            xT = pers.tile([DK, HL, S], BF16, tag="xT", name="xT")
            nc.vector.memset(vx0[:, :, DK], 1.0)
            nc.vector.memset(vx1[:, :, DK], 1.0)

            # ---------------- phase 1: projections ----------------
            for sb in range(NSB):
                s0 = sb * SB
                qs = pin.tile([P, DO, SB], BF16, tag="qs", name="qs")
                nc.sync.dma_start(qs[:], qt_r[:, :, s0 : s0 + SB])
                ks = pin.tile([P, DO, SB], BF16, tag="ks", name="ks")
                nc.sync.dma_start(ks[:], kt_r[:, :, s0 : s0 + SB])
                vs = pin.tile([P, DO, SB], BF16, tag="vs", name="vs")
                nc.sync.dma_start(vs[:], vt_r[:, :, s0 : s0 + SB])

                # q/k projections: psum slices [64, 512] all on partitions 0:64
                pq = psA.tile([P, 4 * SB], F32, tag="sA", name="pq")
                pk = psB.tile([P, 2 * SB], F32, tag="sB", name="pk")
                for h in range(HL):
                    pqh = pq[0:DK, h * SB : (h + 1) * SB]
                    pkh = pk[0:DK, h * SB : (h + 1) * SB]
                    for o in range(DO):
                        nc.tensor.matmul(
                            pqh, wqk_sb[:, o, 0, h, :], qs[:, o],
                            start=(o == 0), stop=(o == DO - 1),
                        )
                    for o in range(DO):
                        nc.tensor.matmul(
                            pkh, wqk_sb[:, o, 1, h, :], ks[:, o],
                            start=(o == 0), stop=(o == DO - 1),
                        )
                    # (q + bq) * 1/sqrt(dk);  (k + bk)
                    nc.vector.tensor_scalar(
                        qT[:, h, s0 : s0 + SB], pqh,
                        bqk_sb[:, h : h + 1], 0.125,
                        mybir.AluOpType.add, mybir.AluOpType.mult,
                    )
                    nc.vector.tensor_scalar(
                        kT[:, h, s0 : s0 + SB], pkh,
                        bqk_sb[:, 2 + h : 3 + h], None,
                        mybir.AluOpType.add,
                    )

                # v directly in [t, j] layout, j = h0 dk | h1 dk
                for tb in range(SB // TB):
                    pv = psV.tile([P, SB], F32, tag="pv", name="pvv")
                    pvv = pv[:, 0:P]
                    for o in range(DO):
                        nc.tensor.matmul(
                            pvv, vs[:, o, tb * TB : (tb + 1) * TB], wv_sb[:, o],
                            start=(o == 0), stop=(o == DO - 1),
                        )
                    tbg = sb * (SB // TB) + tb
                    nc.vector.tensor_copy(vx0[:, tbg, 0:DK], pvv[:, 0:DK])
                    nc.vector.tensor_copy(vx1[:, tbg, 0:DK], pvv[:, DK:P])

            # -------- phase 2: attention + output projection, fused per s-block -----
            for sb in range(NSB):
                s0 = sb * SB
                for h in range(HL):
                    vx = vx0 if h == 0 else vx1
                    oT = psO.tile([DK + 1, SB], F32, tag="oT", name="oT")
                    tb0 = 0
                    for g in GROUPS:
                        pool, width = (psB, 2 * SB) if g == 2 else (psA, 4 * SB)
                        sc = pool.tile([P, width], F32, tag=("sB" if g == 2 else "sA"), name="sc")
                        for i in range(g):
                            tb = tb0 + i
                            nc.tensor.matmul(
                                sc[:, i * SB : (i + 1) * SB],
                                kT[:, h, tb * TB : (tb + 1) * TB],
                                qT[:, h, s0 : s0 + SB],
                                start=True, stop=True,
                            )
                        pt = pP.tile([P, 4 * SB], BF16, tag="P", name="pt")
                        nc.scalar.activation(
                            pt[:, : g * SB], sc[:, : g * SB],
                            mybir.ActivationFunctionType.Exp,
                        )
                        for i in range(g):
                            tb = tb0 + i
                            nc.tensor.matmul(
                                oT, vx[:, tb, :], pt[:, i * SB : (i + 1) * SB],
                                start=(tb == 0), stop=(tb == NTB - 1),
                            )
                        tb0 += g

                    # r = 1/l ; broadcast over 64 partitions ; xT = oT * r
                    r_t = prr.tile([1, SB], F32, tag="r", name="r_t")
                    r_s = prr.tile([1, SB], F32, tag="rs", name="r_s")
                    l_t = prr.tile([1, SB], F32, tag="lt", name="l_t")
                    # custom-DVE reciprocal mis-reads PSUM operands: stage l in SBUF
                    nc.vector.tensor_copy(l_t[:], oT[DK : DK + 1, :])
                    nc.vector.reciprocal_approx_accurate(r_t[:], l_t[:], r_s[:])
                    bc = psV.tile([DK, SB], F32, tag="pv", name="bc")
                    nc.tensor.matmul(bc, ones_sb[:], r_t[:], start=True, stop=True)
                    # DVE has a single PSUM read port: stage the broadcast in SBUF
                    bc_sb = prr.tile([DK, SB], F32, tag="bcs", name="bc_sb")
                    nc.vector.tensor_copy(bc_sb[:], bc)
                    nc.vector.tensor_tensor(
                        xT[:, h, s0 : s0 + SB], oT[0:DK, :], bc_sb[:], mybir.AluOpType.mult
                    )

                # output projection for this s-block
                for si in range(SB // P):
                    so = sb * (SB // P) + si
                    for mb in range(2):
                        m0 = mb * SB
                        op = psV.tile([P, SB], F32, tag="pv", name="op")
                        for h in range(HL):
                            nc.tensor.matmul(
                                op, xT[:, h, so * P : (so + 1) * P],
                                wot_sb[:, h, m0 : m0 + SB],
                                start=(h == 0), stop=(h == HL - 1),
                            )
                        ob = pout.tile([P, SB], F32, tag="ob", name="ob")
                        nc.vector.tensor_copy(ob[:], op)
                        nc.sync.dma_start(out_r[:, so, m0 : m0 + SB], ob[:])

        if loop_n > 0:
            with tc.For_i(0, loop_n, 1):
                body()
        else:
            for _ in range(reps):
                body()

    nc.finalize()
    return nc


def _pack_core_inputs(c, QT, KT, VT, Wq, bq, Wk, bk, Wv, Wo):
    """Per-core input dict (core c owns heads 2c, 2c+1)."""
    h0 = HL * c
    wq = Wq[h0 : h0 + HL].reshape(HL, DO, P, DK).transpose(2, 1, 0, 3)  # [p,o,h,dk]
    wk = Wk[h0 : h0 + HL].reshape(HL, DO, P, DK).transpose(2, 1, 0, 3)
    wqk = np.stack([wq, wk], axis=2).astype(BF16_NP)  # [p, o, qk, h, dk]
    wv = (
        Wv[h0 : h0 + HL].reshape(HL, DO, P, DK).transpose(2, 1, 0, 3).reshape(P, DO, P)
    ).astype(BF16_NP)
    wot = (
        Wo[:, h0 * DK : (h0 + HL) * DK].reshape(D, HL, DK).transpose(2, 1, 0)
    ).astype(BF16_NP)  # [dk, h, m]
    bqk = np.stack(
        [bq[h0], bq[h0 + 1], bk[h0], bk[h0 + 1]], axis=1
    ).astype(np.float32)  # [64, 4]
    return {
        "qt": QT, "kt": KT, "vt": VT,
        "wqk": np.ascontiguousarray(wqk),
        "wv": np.ascontiguousarray(wv),
        "wot": np.ascontiguousarray(wot),
        "bqk": np.ascontiguousarray(bqk),
    }


def make_in_maps(Q, K, V, Wq, bq, Wk, bk, Wv, bv, Wo, bo):
    QT = np.ascontiguousarray(Q.T).astype(BF16_NP)
    KT = np.ascontiguousarray(K.T).astype(BF16_NP)
    VT = np.ascontiguousarray(V.T).astype(BF16_NP)
    return [
        _pack_core_inputs(c, QT, KT, VT, Wq, bq, Wk, bk, Wv, Wo) for c in range(NC)
    ]


def host_combine(partials, Wq, bv, Wo, bo):
    total = np.zeros((S, D), np.float32)
    for p in partials:
        total += p
    # v-bias passes through softmax exactly as +bv on the concat features
    total += bv.reshape(-1).astype(np.float32) @ Wo.T.astype(np.float32) + bo
    return total


_NC_CACHE = {}


def _get_nc(reps=1):
    if reps not in _NC_CACHE:
        _NC_CACHE[reps] = build_nc(reps)
    return _NC_CACHE[reps]


def kernel(Q, K, V, Wq, bq, Wk, bk, Wv, bv, Wo, bo):
    args = [np.asarray(x) for x in (Q, K, V, Wq, bq, Wk, bk, Wv, bv, Wo, bo)]
    Q, K, V, Wq, bq, Wk, bk, Wv, bv, Wo, bo = args
    nc = _get_nc()
    in_maps = make_in_maps(Q, K, V, Wq, bq, Wk, bk, Wv, bv, Wo, bo)
    res = run_bass_kernel_spmd(nc, in_maps, core_ids=list(range(NC)))
    partials = [res.results[c]["out"] for c in range(NC)]
    return host_combine(partials, Wq, bv, Wo, bo)
